# revision 24
# baseline (speedup 1.0000x reference)
"""Trainium2 Bass kernel for CrossFrameSimilarityRefiner.

Computation (per batch element b, fully batch-parallel -> B=8 sharded over 8 cores):
  f = features[:, b]                      # [T, C, P]  T=16, C=256, P=1024
  ss[t,p] = sum_c f^2 ; sm[t,p] = sum_c f ; gm[t,p] = sum_c (f>0)
  S[t,p]  = sm / sqrt(ss)                 # == sum/||.|| (eps clamp irrelevant for randn)
  M'[s,p] = gm  (affine transform of mean(sign(f)) -> identical per-row ranking)
  scores[t,s] = sum_p S[t,p] * M'[s,p]    # row-wise ranking == reference ranking
  mask diag, top-3 indices s* ; compressed c* = s* - (s* > t)   (reference's faithful bug:
  c* indexes the ORIGINAL frame axis)
  out[t] = (W/3) @ (f[c*0]+f[c*1]+f[c*2]) + b

v2 layout: features converted to fp16 on the HOST (sign-preserving for
denormal-flush cases so is_gt matches fp32 semantics) and DMA'd in at half
the fp32 traffic; output written fp16 and upconverted on host. One
persistent c-major fp16 SBUF tensor [128, T*CC*P] feeds everything:
per-frame stats run as 3 col-tiled concurrent one-hot matmuls (strips
0/32/64) chased by the per-frame DMAs; top-k via DVE max8/max_index; the
gather+linear phase splits work between the PE (PSUM-accumulated
W @ f[src] matmuls with register-indexed dynamic rhs slices) and the DVE
(fp16 adds) to balance engine time, with the bias fused into the ACT psum
evacuation and fp16 output DMAs.
"""

import numpy as np

import concourse.bacc as bacc
import concourse.bass as bass
import concourse.tile as tile
from concourse import mybir
from concourse.bass_utils import run_bass_kernel_spmd

FP32 = mybir.dt.float32
F16 = mybir.dt.float16
I32 = mybir.dt.int32
U32 = mybir.dt.uint32
AF = mybir.ActivationFunctionType
OP = mybir.AluOpType
ET = mybir.EngineType

N_CORES = 8
BIG = 1.0e30

# phase-C work split: frames whose 3-source gather-sum runs fully on the PE
# (3 accumulating matmul streams), half on the PE (1 DVE add + 2 streams),
# or fully on the DVE (2 adds + 1 stream).  Balances PE vs DVE busy time.
PE_FULL_FRAMES = (5,)
PE_HALF_FRAMES = (11,)
DVE_EVAC = set()       # (frame, dc) pairs whose psum evac runs on the DVE


def _emit(nc, tc, T, C, P, K, handles):
    feat_h = handles["features"]
    out_h = handles["out"]
    sdbg_h = handles["scores_dbg"]
    idbg_h = handles["idx_dbg"]
    CC = C // 128          # c chunks (2)
    PH = P // 512          # psum-width chunks of p (2)
    PB = P // 128          # 128-blocks of p (8)
    DC = C // 128          # d chunks for output (2)
    CCP = CC * P           # 2048 (free elems per frame per partition)

    with tc.tile_pool(name="persist", bufs=1) as pp:
        # ---- constants (from DRAM inputs) ----
        wt3_sb = pp.tile([128, CC, C], F16, tag="wt3")
        bcol_sb = pp.tile([128, DC], FP32, tag="bcol")
        esel_sb = pp.tile([128, T * T], F16, tag="esel")
        i16_sb = pp.tile([96, T], FP32, tag="i16")
        diag_sb = pp.tile([T, T], FP32, tag="diag")
        tcol_sb = pp.tile([T, 1], FP32, tag="tcol")
        # esel is needed immediately by the stats matmuls; i16 by phase B
        # (frame 0's feature DMA is dispatched first in phase A)
        pending_consts = [(esel_sb, "esel"), (i16_sb, "i16")]

        # ---- persistent state ----
        f16_sb = pp.tile([128, T * CCP], F16, tag="f16")
        # stats rows: sm at partitions 0..15, ss at 32..47, gm at 64..79
        stats_sb = pp.tile([96, P], FP32, tag="stats")
        sm_sb = stats_sb[0:T, :]
        ss_sb = stats_sb[32:32 + T, :]
        gm_sb = stats_sb[64:64 + T, :]
        smt_sb = pp.tile([128, PB, T], FP32, tag="smT")
        rnt_sb = pp.tile([128, PB, T], FP32, tag="rnT")
        rst_sb = pp.tile([128, PB, T], FP32, tag="rsT")
        spt_sb = pp.tile([128, PB, T], FP32, tag="SpT")
        mpt_sb = pp.tile([128, PB, T], FP32, tag="MpT")
        scores_sb = pp.tile([T, T], FP32, tag="scores")
        maxv_sb = pp.tile([T, 8], FP32, tag="maxv")
        maxi_sb = pp.tile([T, 8], U32, tag="maxi")
        idxf_sb = pp.tile([T, K], FP32, tag="idxf")
        gt_sb = pp.tile([T, K], FP32, tag="gt")
        cidxf_sb = pp.tile([T, K], FP32, tag="cidxf")
        cidx_sb = pp.tile([T, K], I32, tag="cidx")
        row_sb = pp.tile([1, T * K], I32, tag="row")

        # ================= Phase A: stream in, stats =================
        with tc.tile_pool(name="statsps", bufs=1, space="PSUM") as sps, \
             tc.tile_pool(name="stream", bufs=4) as sp:
            # per (stat, p-half) psum banks; stat j writes partition strip 32j
            # so the 3 stats' matmuls col-tile and run concurrently on the PE
            st_ps = [[sps.tile([96, 512], FP32, tag=f"stp{ph}_{j}",
                               name=f"stp{ph}_{j}") for j in range(3)]
                     for ph in range(PH)]

            last_sq = None
            for t in range(T):
                fch = f16_sb[:, t * CCP:(t + 1) * CCP]
                nc.sync.dma_start(fch, feat_h[t])
                if t == 0:
                    for t_, name in pending_consts:
                        nc.sync.dma_start(t_[:], handles[name].ap())
                sq = sp.tile([128, CCP], F16, tag="sq")
                gsc = sp.tile([128, CCP], F16, tag="gsc")
                # squares alternate ACT/DVE so neither engine paces phase A
                if t % 2 == 1:
                    nc.scalar.activation(sq[:], fch, AF.Square)
                    last_sq = sq
                else:
                    nc.vector.tensor_mul(sq[:], fch, fch)
                nc.vector.tensor_scalar(gsc[:], fch, 0.0, None, OP.is_gt)
                st = (t == 0)
                sx = (t == T - 1)
                lhs = esel_sb[:, T * t:T * (t + 1)]
                for cc in range(CC):
                    for ph in range(PH):
                        o = cc * P + ph * 512
                        srcs = (f16_sb[:, t * CCP + o:t * CCP + o + 512],
                                sq[:, o:o + 512], gsc[:, o:o + 512])
                        for j, src in enumerate(srcs):
                            nc.tensor.matmul(
                                st_ps[ph][j][32 * j:32 * j + T, :], lhs, src,
                                start=st and cc == 0, stop=sx and cc == CC - 1,
                                tile_position=(0, 32 * j))

            # preload the Sqrt ACT table while the stats evacuate (depends on
            # the last Square so it cannot run early and evict its table)
            dummy_sb = sp.tile([1, 1], FP32, tag="dummy")
            nc.scalar.activation(dummy_sb[:], last_sq[0:1, 0:1], AF.Sqrt)

            # ss evacuates through ACT with fused sqrt (-> rn); sm/gm copy
            # out on the DVE in parallel
            for ph in range(PH):
                sl = slice(ph * 512, (ph + 1) * 512)
                nc.scalar.activation(stats_sb[32:32 + T, sl],
                                     st_ps[ph][1][32:32 + T, :], AF.Sqrt)
            for ph in range(PH):
                sl = slice(ph * 512, (ph + 1) * 512)
                for j in (0, 2):
                    nc.vector.tensor_copy(
                        stats_sb[32 * j:32 * j + T, sl],
                        st_ps[ph][j][32 * j:32 * j + T, :])

        # remaining constants (needed from phase B onward)
        for name, t_ in (("wt3", wt3_sb), ("bcol", bcol_sb),
                         ("diagbig", diag_sb), ("tcol", tcol_sb)):
            nc.sync.dma_start(t_[:], handles[name].ap())

        # ================= Phase B: scores + top-k =================
        with tc.tile_pool(name="bps", bufs=2, space="PSUM") as bps:
            # transpose raw stats to p-major; the ss strip already holds
            # rn = sqrt(ss), so only recip + mul remain, on 128 partitions
            for src, ibase, dst in ((ss_sb, 32, rnt_sb), (sm_sb, 0, smt_sb),
                                    (gm_sb, 64, mpt_sb)):
                tr = bps.tile([128, PB * T], FP32, tag="tr", name="tr")
                ident = i16_sb[ibase:ibase + T, :]
                for pb in range(PB):
                    nc.tensor.transpose(tr[:, pb * T:(pb + 1) * T],
                                        src[:, pb * 128:(pb + 1) * 128], ident)
                nc.scalar.copy(dst[:, :, :], tr[:])
            nc.vector.reciprocal(rst_sb[:], rnt_sb[:])
            nc.vector.tensor_mul(spt_sb[:], smt_sb[:], rst_sb[:])

            sc_ps = bps.tile([T, T], FP32, tag="scps")
            for pb in range(PB):
                nc.tensor.matmul(sc_ps[:], spt_sb[:, pb, :], mpt_sb[:, pb, :],
                                 start=(pb == 0), stop=(pb == PB - 1))
            # exclude s == t, move to SBUF
            nc.vector.tensor_sub(scores_sb[:], sc_ps[:], diag_sb[:])

            nc.vector.max(maxv_sb[:], scores_sb[:])
            nc.vector.max_index(maxi_sb[:], maxv_sb[:], scores_sb[:])
            # compressed index c* = s* - (s* > t)   (faithful reference bug)
            nc.vector.tensor_scalar(gt_sb[:], maxi_sb[:, 0:K], tcol_sb[:, 0:1], None, OP.is_gt)
            nc.vector.tensor_sub(cidx_sb[:], maxi_sb[:, 0:K], gt_sb[:])
            nc.sync.dma_start(row_sb[:], cidx_sb[:])
            # debug outputs
            nc.sync.dma_start(sdbg_h.ap(), scores_sb[:])
            nc.sync.dma_start(idbg_h.ap(), row_sb[:])

            # HAM keep-warm pepper: tiny throwaway matmuls gated on
            # progressively later products keep every PE idle gap under the
            # ~3.4us MID re-throttle window until the real phase-C matmuls
            wps = bps.tile([16, 512], FP32, tag="warm", bufs=1)
            nc.tensor.matmul(wps[:, 0:8], i16_sb[0:T, :], maxv_sb[:],
                             start=True, stop=True)
            nc.tensor.matmul(wps[:, 0:K], i16_sb[0:T, :], gt_sb[:],
                             start=True, stop=True)

            # batched register loads for the T*K gather offsets; emitted here
            # (gated only on row_sb) so phase C's first adds start ASAP.
            # Small DVE batches so the first frames' adds lead the rest.
            avals = []
            step = T * K // 4
            for lo in range(0, T * K, step):
                _, v = nc.values_load_multi_w_load_instructions(
                    row_sb[0:1, lo:lo + step],
                    engines=bass.OrderedSet([ET.DVE]),
                    min_val=0, max_val=T - 2,
                    skip_runtime_bounds_check=True,
                )
                avals.extend(v)
            pevals_all = []
            for lo in range(0, T * K, T * K // 2):
                _, v = nc.values_load_multi_w_load_instructions(
                    row_sb[0:1, lo:lo + T * K // 2],
                    engines=bass.OrderedSet([ET.PE]),
                    min_val=0, max_val=T - 2,
                    skip_runtime_bounds_check=True,
                )
                pevals_all.extend(v)
            pevals = {}
            for t in PE_FULL_FRAMES:
                pevals[t] = [pevals_all[K * t + k] for k in range(K)]
            for t in PE_HALF_FRAMES:
                pevals[t] = [pevals_all[K * t + K - 1]]
            # pepper3: gated on the PE register loads, covers the gap until
            # the first gather matmul
            nc.tensor.matmul(
                wps[:], esel_sb[:, 0:T],
                f16_sb[:, bass.ds(pevals_all[0] * CCP, 512)],
                start=True, stop=True)

        # ================= Phase C: gather-combine + linear =================
        with tc.tile_pool(name="cps", bufs=4, space="PSUM") as cps, \
             tc.tile_pool(name="cpool", bufs=3) as cp:
            def evac(t, dc, po, osb):
                if (t, dc) in DVE_EVAC:
                    nc.vector.tensor_scalar(osb[:, dc, :], po[:],
                                            bcol_sb[:, dc:dc + 1], None, OP.add)
                else:
                    nc.scalar.activation(osb[:, dc, :], po[:], AF.Identity,
                                         bias=bcol_sb[:, dc:dc + 1])

            for t in range(T):
                vals = avals[K * t:K * (t + 1)]
                osb = cp.tile([128, DC, P], F16, tag="osb", bufs=4)
                if t in PE_FULL_FRAMES:
                    # all 3 sources streamed straight into the PSUM accum
                    pv = pevals[t]
                    for dc in range(DC):
                        po = cps.tile([128, P], FP32, tag="po")
                        for cc in range(CC):
                            lw = wt3_sb[:, cc, dc * 128:(dc + 1) * 128]
                            for ph in range(PH):
                                o = cc * P + ph * 512
                                for k in range(K):
                                    nc.tensor.matmul(
                                        po[:, ph * 512:(ph + 1) * 512], lw,
                                        f16_sb[:, bass.ds(pv[k] * CCP + o, 512)],
                                        start=(cc == 0 and k == 0),
                                        stop=(cc == CC - 1 and k == K - 1))
                        evac(t, dc, po, osb)
                elif t in PE_HALF_FRAMES:
                    # one DVE add (src0+src1); PE streams the pair-sum + src2
                    pv = pevals[t]
                    mf2 = cp.tile([128, CCP], F16, tag="mf2")
                    a0 = f16_sb[:, bass.ds(vals[0] * CCP, CCP)]
                    a1 = f16_sb[:, bass.ds(vals[1] * CCP, CCP)]
                    nc.vector.tensor_add(mf2[:], a0, a1)
                    for dc in range(DC):
                        po = cps.tile([128, P], FP32, tag="po")
                        for cc in range(CC):
                            lw = wt3_sb[:, cc, dc * 128:(dc + 1) * 128]
                            for ph in range(PH):
                                o = cc * P + ph * 512
                                srcs = (mf2[:, o:o + 512],
                                        f16_sb[:, bass.ds(pv[0] * CCP + o, 512)])
                                for k, src in enumerate(srcs):
                                    nc.tensor.matmul(
                                        po[:, ph * 512:(ph + 1) * 512], lw, src,
                                        start=(cc == 0 and k == 0),
                                        stop=(cc == CC - 1 and k == 1))
                        evac(t, dc, po, osb)
                else:
                    # gather-sum on the DVE, then one W stream per d-chunk
                    mf16 = cp.tile([128, CCP], F16, tag="mf16")
                    a0 = f16_sb[:, bass.ds(vals[0] * CCP, CCP)]
                    a1 = f16_sb[:, bass.ds(vals[1] * CCP, CCP)]
                    nc.vector.tensor_add(mf16[:], a0, a1)
                    for k in range(2, K):
                        ak = f16_sb[:, bass.ds(vals[k] * CCP, CCP)]
                        nc.vector.tensor_add(mf16[:], mf16[:], ak)
                    for dc in range(DC):
                        po = cps.tile([128, P], FP32, tag="po")
                        for cc in range(CC):
                            lw = wt3_sb[:, cc, dc * 128:(dc + 1) * 128]
                            for ph in range(PH):
                                o = cc * P + ph * 512
                                nc.tensor.matmul(
                                    po[:, ph * 512:(ph + 1) * 512], lw,
                                    mf16[:, o:o + 512],
                                    start=(cc == 0), stop=(cc == CC - 1))
                        evac(t, dc, po, osb)
                nc.sync.dma_start(out_h[t], osb[:])


def build_program(T=16, C=256, P=1024, K=3):
    nc = bacc.Bacc("TRN2", target_bir_lowering=False, debug=False,
                   num_devices=N_CORES)
    CC = C // 128
    handles = {}
    handles["features"] = nc.dram_tensor("features", [T, 128, CC * P], F16,
                                         kind="ExternalInput")
    for name, shape, dt in (
        ("wt3", [128, C // 128, C], F16),
        ("bcol", [128, C // 128], FP32),
        ("esel", [128, T * T], F16),
        ("i16", [96, T], FP32),
        ("diagbig", [T, T], FP32),
        ("tcol", [T, 1], FP32),
    ):
        handles[name] = nc.dram_tensor(name, shape, dt, kind="ExternalInput")
    handles["out"] = nc.dram_tensor("out", [T, 128, C // 128, P], F16,
                                    kind="ExternalOutput")
    handles["scores_dbg"] = nc.dram_tensor("scores_dbg", [T, T], FP32,
                                           kind="ExternalOutput")
    handles["idx_dbg"] = nc.dram_tensor("idx_dbg", [1, T * K], I32,
                                        kind="ExternalOutput")

    with tile.TileContext(nc) as tc:
        _emit(nc, tc, T, C, P, K, handles)
    nc.compile()
    return nc


def _host_consts(W, b, T, C, K):
    consts = {}
    wt3 = (np.asarray(W, np.float32).T / float(K)).astype(np.float32)  # [C, C] (c, d)
    # [c_in(partition), cc, d] in fp16
    w4 = wt3.reshape(C // 128, 128, C).transpose(1, 0, 2)
    consts["wt3"] = np.ascontiguousarray(w4.astype(np.float16))
    consts["bcol"] = np.ascontiguousarray(
        np.asarray(b, np.float32).reshape(C // 128, 128).T)
    esel = np.zeros((128, T * T), np.float16)
    for t in range(T):
        esel[:, T * t + t] = 1.0
    consts["esel"] = esel
    i16 = np.zeros((96, T), np.float32)
    for r in (0, 32, 64):
        i16[r:r + T, :] = np.eye(T, dtype=np.float32)
    consts["i16"] = i16
    consts["diagbig"] = (np.eye(T, dtype=np.float32) * BIG).astype(np.float32)
    consts["tcol"] = np.arange(T, dtype=np.float32).reshape(T, 1)
    return consts


def _prep_features(features):
    """fp32 [T,B,C,H,W] -> per-core fp16 dev arrays [T, 128, CC*P].

    fp16 conversion preserves is_gt(f, 0) semantics: fp32 values tiny enough
    to round to +-0 are clamped to the smallest fp16 subnormal with the
    original sign (so the sign-count statistic matches the fp32 reference).
    """
    f = np.asarray(features, np.float32)
    T, B, C, H, Wd = f.shape
    P = H * Wd
    CC = C // 128
    f16 = f.astype(np.float16)
    tiny = (f16 == 0) & (f != 0)
    if tiny.any():
        f16 = f16.copy()
        f16[tiny] = np.copysign(np.float16(6e-8), f[tiny]).astype(np.float16)
    fr = f16.reshape(T, B, CC, 128, P)
    return [np.ascontiguousarray(
        fr[:, i].transpose(0, 2, 1, 3).reshape(T, 128, CC * P))
        for i in range(B)]


_CACHE = {}


def kernel(features, W, b, top_k):
    features = np.asarray(features, np.float32)
    T, B, C, H, Wd = features.shape
    P = H * Wd
    K = int(top_k)
    assert B == N_CORES and C == 256 and P == 1024 and T == 16 and K == 3

    key = (T, C, P, K)
    if key not in _CACHE:
        _CACHE[key] = build_program(T, C, P, K)
    nc = _CACHE[key]

    consts = _host_consts(W, b, T, C, K)
    percore = _prep_features(features)
    in_maps = [{"features": percore[i], **consts} for i in range(N_CORES)]
    res = run_bass_kernel_spmd(nc, in_maps, list(range(N_CORES)))
    # out dev layout [T, 128, DC, P] fp16, d = dc*128 + partition
    outs = []
    for i in range(N_CORES):
        o = np.asarray(res.results[i]["out"])
        outs.append(o.transpose(0, 2, 1, 3).reshape(T, C, P).astype(np.float32))
    out = np.stack(outs, axis=1)
    return np.ascontiguousarray(out.reshape(T, B, C, H, Wd))


# revision 26
# speedup vs baseline: 1.0365x; 1.0365x over previous
"""Trainium2 Bass kernel for CrossFrameSimilarityRefiner.

Computation (per batch element b, fully batch-parallel -> B=8 sharded over 8 cores):
  f = features[:, b]                      # [T, C, P]  T=16, C=256, P=1024
  ss[t,p] = sum_c f^2 ; sm[t,p] = sum_c f ; gm[t,p] = sum_c (f>0)
  S[t,p]  = sm / sqrt(ss)                 # == sum/||.|| (eps clamp irrelevant for randn)
  M'[s,p] = gm  (affine transform of mean(sign(f)) -> identical per-row ranking)
  scores[t,s] = sum_p S[t,p] * M'[s,p]    # row-wise ranking == reference ranking
  mask diag, top-3 indices s* ; compressed c* = s* - (s* > t)   (reference's faithful bug:
  c* indexes the ORIGINAL frame axis)
  out[t] = (W/3) @ (f[c*0]+f[c*1]+f[c*2]) + b

Design (measured ~96us/core on TRN2, vs ~120-140us for the fp32 baseline):
- fp16 I/O: features are converted to fp16 on the host (sign-preserving for
  values that would round to +-0, so is_gt matches fp32 semantics) halving
  the input DMA; the output is written fp16 and upconverted on the host.
  Index margins were verified numerically: fp16 stats reproduce the fp32
  reference's top-3 sets exactly (0/128 flips); fp8 anywhere would not.
- Phase A streams one [128, 4KB] DMA per frame into a persistent c-major
  fp16 SBUF tensor; per-frame squares alternate ACT/DVE (ACT runs at 1x so
  a single engine would pace the phase), is_gt on DVE, and the three stats
  accumulate via one-hot fp16 matmuls col-tiled to concurrent 32-partition
  strips (0/32/64) so the PE streams all three per-frame slices at once.
- Phase B: PE transposes the stats to p-major, Sqrt is fused into the ss
  PSUM evacuation, reciprocal+mul on DVE, a tiny [T,T] scores matmul, then
  top-k via DVE max8/max_index and the compressed-index arithmetic.
- Phase C gathers with register-indexed dynamic SBUF slices: most frames sum
  3 source frames with two DVE fp16 adds then run W through the PE; one
  frame runs fully in PE PSUM accumulation and one half (balancing DVE vs
  PE busy time); bias is fused into the ACT psum evacuation (fp16 out) and
  each frame leaves via a single [128, 4KB] DMA.
"""

import numpy as np

import concourse.bacc as bacc
import concourse.bass as bass
import concourse.tile as tile
from concourse import mybir
from concourse.bass_utils import run_bass_kernel_spmd

FP32 = mybir.dt.float32
F16 = mybir.dt.float16
I32 = mybir.dt.int32
U32 = mybir.dt.uint32
AF = mybir.ActivationFunctionType
OP = mybir.AluOpType
ET = mybir.EngineType

N_CORES = 8
BIG = 1.0e30

# phase-C work split: frames whose 3-source gather-sum runs fully on the PE
# (3 accumulating matmul streams), half on the PE (1 DVE add + 2 streams),
# or fully on the DVE (2 adds + 1 stream).  Balances PE vs DVE busy time.
PE_FULL_FRAMES = (5,)
PE_HALF_FRAMES = (11,)
DVE_EVAC = set()       # (frame, dc) pairs whose psum evac runs on the DVE


def _emit(nc, tc, T, C, P, K, handles):
    feat_h = handles["features"]
    out_h = handles["out"]
    sdbg_h = handles["scores_dbg"]
    idbg_h = handles["idx_dbg"]
    CC = C // 128          # c chunks (2)
    PH = P // 512          # psum-width chunks of p (2)
    PB = P // 128          # 128-blocks of p (8)
    DC = C // 128          # d chunks for output (2)
    CCP = CC * P           # 2048 (free elems per frame per partition)

    with tc.tile_pool(name="persist", bufs=1) as pp:
        # ---- constants (from DRAM inputs) ----
        wt3_sb = pp.tile([128, CC, C], F16, tag="wt3")
        bcol_sb = pp.tile([128, DC], FP32, tag="bcol")
        esel_sb = pp.tile([128, T * T], F16, tag="esel")
        i16_sb = pp.tile([96, T], FP32, tag="i16")
        diag_sb = pp.tile([T, T], FP32, tag="diag")
        tcol_sb = pp.tile([T, 1], FP32, tag="tcol")
        # esel is needed immediately by the stats matmuls; i16 by phase B
        nc.sync.dma_start(esel_sb[:], handles["esel"].ap())
        nc.sync.dma_start(i16_sb[:], handles["i16"].ap())

        # ---- persistent state ----
        f16_sb = pp.tile([128, T * CCP], F16, tag="f16")
        # stats rows: sm at partitions 0..15, ss at 32..47, gm at 64..79
        stats_sb = pp.tile([96, P], FP32, tag="stats")
        sm_sb = stats_sb[0:T, :]
        ss_sb = stats_sb[32:32 + T, :]
        gm_sb = stats_sb[64:64 + T, :]
        smt_sb = pp.tile([128, PB, T], FP32, tag="smT")
        rnt_sb = pp.tile([128, PB, T], FP32, tag="rnT")
        rst_sb = pp.tile([128, PB, T], FP32, tag="rsT")
        spt_sb = pp.tile([128, PB, T], FP32, tag="SpT")
        mpt_sb = pp.tile([128, PB, T], FP32, tag="MpT")
        scores_sb = pp.tile([T, T], FP32, tag="scores")
        maxv_sb = pp.tile([T, 8], FP32, tag="maxv")
        maxi_sb = pp.tile([T, 8], U32, tag="maxi")
        idxf_sb = pp.tile([T, K], FP32, tag="idxf")
        gt_sb = pp.tile([T, K], FP32, tag="gt")
        cidxf_sb = pp.tile([T, K], FP32, tag="cidxf")
        cidx_sb = pp.tile([T, K], I32, tag="cidx")
        row_sb = pp.tile([1, T * K], I32, tag="row")

        # ================= Phase A: stream in, stats =================
        with tc.tile_pool(name="statsps", bufs=1, space="PSUM") as sps, \
             tc.tile_pool(name="stream", bufs=4) as sp:
            # per (stat, p-half) psum banks; stat j writes partition strip 32j
            # so the 3 stats' matmuls col-tile and run concurrently on the PE
            st_ps = [[sps.tile([96, 512], FP32, tag=f"stp{ph}_{j}",
                               name=f"stp{ph}_{j}") for j in range(3)]
                     for ph in range(PH)]

            last_sq = None
            for t in range(T):
                fch = f16_sb[:, t * CCP:(t + 1) * CCP]
                nc.sync.dma_start(fch, feat_h[t])
                sq = sp.tile([128, CCP], F16, tag="sq")
                gsc = sp.tile([128, CCP], F16, tag="gsc")
                # squares alternate ACT/DVE so neither engine paces phase A
                if t % 2 == 0:
                    nc.scalar.activation(sq[:], fch, AF.Square)
                    last_sq = sq
                else:
                    nc.vector.tensor_mul(sq[:], fch, fch)
                nc.vector.tensor_scalar(gsc[:], fch, 0.0, None, OP.is_gt)
                st = (t == 0)
                sx = (t == T - 1)
                lhs = esel_sb[:, T * t:T * (t + 1)]
                for cc in range(CC):
                    for ph in range(PH):
                        o = cc * P + ph * 512
                        srcs = (f16_sb[:, t * CCP + o:t * CCP + o + 512],
                                sq[:, o:o + 512], gsc[:, o:o + 512])
                        for j, src in enumerate(srcs):
                            nc.tensor.matmul(
                                st_ps[ph][j][32 * j:32 * j + T, :], lhs, src,
                                start=st and cc == 0, stop=sx and cc == CC - 1,
                                tile_position=(0, 32 * j))

            # preload the Sqrt ACT table while the stats evacuate (depends on
            # the last Square so it cannot run early and evict its table)
            dummy_sb = sp.tile([1, 1], FP32, tag="dummy")
            nc.scalar.activation(dummy_sb[:], last_sq[0:1, 0:1], AF.Sqrt)

            # ss evacuates through ACT with fused sqrt (-> rn); sm/gm copy
            # out on the DVE in parallel
            for ph in range(PH):
                sl = slice(ph * 512, (ph + 1) * 512)
                nc.scalar.activation(stats_sb[32:32 + T, sl],
                                     st_ps[ph][1][32:32 + T, :], AF.Sqrt)
            for ph in range(PH):
                sl = slice(ph * 512, (ph + 1) * 512)
                for j in (0, 2):
                    nc.vector.tensor_copy(
                        stats_sb[32 * j:32 * j + T, sl],
                        st_ps[ph][j][32 * j:32 * j + T, :])

        # remaining constants (needed from phase B onward)
        for name, t_ in (("wt3", wt3_sb), ("bcol", bcol_sb),
                         ("diagbig", diag_sb), ("tcol", tcol_sb)):
            nc.sync.dma_start(t_[:], handles[name].ap())

        # ================= Phase B: scores + top-k =================
        with tc.tile_pool(name="bps", bufs=2, space="PSUM") as bps:
            # transpose raw stats to p-major; the ss strip already holds
            # rn = sqrt(ss), so only recip + mul remain, on 128 partitions
            for src, ibase, dst in ((ss_sb, 32, rnt_sb), (sm_sb, 0, smt_sb),
                                    (gm_sb, 64, mpt_sb)):
                tr = bps.tile([128, PB * T], FP32, tag="tr", name="tr")
                ident = i16_sb[ibase:ibase + T, :]
                for pb in range(PB):
                    nc.tensor.transpose(tr[:, pb * T:(pb + 1) * T],
                                        src[:, pb * 128:(pb + 1) * 128], ident)
                nc.scalar.copy(dst[:, :, :], tr[:])
            nc.vector.reciprocal(rst_sb[:], rnt_sb[:])
            nc.vector.tensor_mul(spt_sb[:], smt_sb[:], rst_sb[:])

            sc_ps = bps.tile([T, T], FP32, tag="scps")
            for pb in range(PB):
                nc.tensor.matmul(sc_ps[:], spt_sb[:, pb, :], mpt_sb[:, pb, :],
                                 start=(pb == 0), stop=(pb == PB - 1))
            # exclude s == t, move to SBUF
            nc.vector.tensor_sub(scores_sb[:], sc_ps[:], diag_sb[:])

            nc.vector.max(maxv_sb[:], scores_sb[:])
            nc.vector.max_index(maxi_sb[:], maxv_sb[:], scores_sb[:])
            # compressed index c* = s* - (s* > t)   (faithful reference bug)
            nc.vector.tensor_copy(idxf_sb[:], maxi_sb[:, 0:K])
            nc.vector.tensor_scalar(gt_sb[:], idxf_sb[:], tcol_sb[:, 0:1], None, OP.is_gt)
            nc.vector.tensor_sub(cidxf_sb[:], idxf_sb[:], gt_sb[:])
            nc.vector.tensor_copy(cidx_sb[:], cidxf_sb[:])
            nc.sync.dma_start(row_sb[:], cidx_sb[:])
            # debug outputs
            nc.sync.dma_start(sdbg_h.ap(), scores_sb[:])
            nc.sync.dma_start(idbg_h.ap(), row_sb[:])

            # tiny PE op dependent on late phase-B data: keeps the HAM
            # activity window from seeing a long PE-idle gap before phase C
            warm_ps = bps.tile([T, K], FP32, tag="warm")
            nc.tensor.matmul(warm_ps[:], i16_sb[0:T, :], gt_sb[:],
                             start=True, stop=True)

        # ================= Phase C: gather-combine + linear =================
        with tc.tile_pool(name="cps", bufs=4, space="PSUM") as cps, \
             tc.tile_pool(name="cpool", bufs=3) as cp:
            # batched register loads for the T*K gather offsets; small batches
            # so the first frames' adds start before all values are loaded
            avals = []
            step = T * K // 4
            for lo in range(0, T * K, step):
                _, v = nc.values_load_multi_w_load_instructions(
                    row_sb[0:1, lo:lo + step],
                    engines=bass.OrderedSet([ET.DVE]),
                    min_val=0, max_val=T - 2,
                    skip_runtime_bounds_check=True,
                )
                avals.extend(v)
            # PE-engine registers for the frames whose gather runs on the PE
            pevals = {}
            for t in PE_FULL_FRAMES:
                _, v = nc.values_load_multi_w_load_instructions(
                    row_sb[0:1, K * t:K * (t + 1)],
                    engines=bass.OrderedSet([ET.PE]),
                    min_val=0, max_val=T - 2,
                    skip_runtime_bounds_check=True,
                )
                pevals[t] = list(v)
            for t in PE_HALF_FRAMES:
                _, v = nc.values_load_multi_w_load_instructions(
                    row_sb[0:1, K * t + K - 1:K * (t + 1)],
                    engines=bass.OrderedSet([ET.PE]),
                    min_val=0, max_val=T - 2,
                    skip_runtime_bounds_check=True,
                )
                pevals[t] = list(v)

            def evac(t, dc, po, osb):
                if (t, dc) in DVE_EVAC:
                    nc.vector.tensor_scalar(osb[:, dc, :], po[:],
                                            bcol_sb[:, dc:dc + 1], None, OP.add)
                else:
                    nc.scalar.activation(osb[:, dc, :], po[:], AF.Identity,
                                         bias=bcol_sb[:, dc:dc + 1])

            for t in range(T):
                vals = avals[K * t:K * (t + 1)]
                osb = cp.tile([128, DC, P], F16, tag="osb", bufs=4)
                if t in PE_FULL_FRAMES:
                    # all 3 sources streamed straight into the PSUM accum
                    pv = pevals[t]
                    for dc in range(DC):
                        po = cps.tile([128, P], FP32, tag="po")
                        for cc in range(CC):
                            lw = wt3_sb[:, cc, dc * 128:(dc + 1) * 128]
                            for ph in range(PH):
                                o = cc * P + ph * 512
                                for k in range(K):
                                    nc.tensor.matmul(
                                        po[:, ph * 512:(ph + 1) * 512], lw,
                                        f16_sb[:, bass.ds(pv[k] * CCP + o, 512)],
                                        start=(cc == 0 and k == 0),
                                        stop=(cc == CC - 1 and k == K - 1))
                        evac(t, dc, po, osb)
                elif t in PE_HALF_FRAMES:
                    # one DVE add (src0+src1); PE streams the pair-sum + src2
                    pv = pevals[t]
                    mf2 = cp.tile([128, CCP], F16, tag="mf2")
                    a0 = f16_sb[:, bass.ds(vals[0] * CCP, CCP)]
                    a1 = f16_sb[:, bass.ds(vals[1] * CCP, CCP)]
                    nc.vector.tensor_add(mf2[:], a0, a1)
                    for dc in range(DC):
                        po = cps.tile([128, P], FP32, tag="po")
                        for cc in range(CC):
                            lw = wt3_sb[:, cc, dc * 128:(dc + 1) * 128]
                            for ph in range(PH):
                                o = cc * P + ph * 512
                                srcs = (mf2[:, o:o + 512],
                                        f16_sb[:, bass.ds(pv[0] * CCP + o, 512)])
                                for k, src in enumerate(srcs):
                                    nc.tensor.matmul(
                                        po[:, ph * 512:(ph + 1) * 512], lw, src,
                                        start=(cc == 0 and k == 0),
                                        stop=(cc == CC - 1 and k == 1))
                        evac(t, dc, po, osb)
                else:
                    # gather-sum on the DVE, then one W stream per d-chunk
                    mf16 = cp.tile([128, CCP], F16, tag="mf16")
                    a0 = f16_sb[:, bass.ds(vals[0] * CCP, CCP)]
                    a1 = f16_sb[:, bass.ds(vals[1] * CCP, CCP)]
                    nc.vector.tensor_add(mf16[:], a0, a1)
                    for k in range(2, K):
                        ak = f16_sb[:, bass.ds(vals[k] * CCP, CCP)]
                        nc.vector.tensor_add(mf16[:], mf16[:], ak)
                    for dc in range(DC):
                        po = cps.tile([128, P], FP32, tag="po")
                        for cc in range(CC):
                            lw = wt3_sb[:, cc, dc * 128:(dc + 1) * 128]
                            for ph in range(PH):
                                o = cc * P + ph * 512
                                nc.tensor.matmul(
                                    po[:, ph * 512:(ph + 1) * 512], lw,
                                    mf16[:, o:o + 512],
                                    start=(cc == 0), stop=(cc == CC - 1))
                        evac(t, dc, po, osb)
                nc.sync.dma_start(out_h[t], osb[:])


def build_program(T=16, C=256, P=1024, K=3):
    nc = bacc.Bacc("TRN2", target_bir_lowering=False, debug=False,
                   num_devices=N_CORES)
    CC = C // 128
    handles = {}
    handles["features"] = nc.dram_tensor("features", [T, 128, CC * P], F16,
                                         kind="ExternalInput")
    for name, shape, dt in (
        ("wt3", [128, C // 128, C], F16),
        ("bcol", [128, C // 128], FP32),
        ("esel", [128, T * T], F16),
        ("i16", [96, T], FP32),
        ("diagbig", [T, T], FP32),
        ("tcol", [T, 1], FP32),
    ):
        handles[name] = nc.dram_tensor(name, shape, dt, kind="ExternalInput")
    handles["out"] = nc.dram_tensor("out", [T, 128, C // 128, P], F16,
                                    kind="ExternalOutput")
    handles["scores_dbg"] = nc.dram_tensor("scores_dbg", [T, T], FP32,
                                           kind="ExternalOutput")
    handles["idx_dbg"] = nc.dram_tensor("idx_dbg", [1, T * K], I32,
                                        kind="ExternalOutput")

    with tile.TileContext(nc) as tc:
        _emit(nc, tc, T, C, P, K, handles)
    nc.compile()
    return nc


def _host_consts(W, b, T, C, K):
    consts = {}
    wt3 = (np.asarray(W, np.float32).T / float(K)).astype(np.float32)  # [C, C] (c, d)
    # [c_in(partition), cc, d] in fp16
    w4 = wt3.reshape(C // 128, 128, C).transpose(1, 0, 2)
    consts["wt3"] = np.ascontiguousarray(w4.astype(np.float16))
    consts["bcol"] = np.ascontiguousarray(
        np.asarray(b, np.float32).reshape(C // 128, 128).T)
    esel = np.zeros((128, T * T), np.float16)
    for t in range(T):
        esel[:, T * t + t] = 1.0
    consts["esel"] = esel
    i16 = np.zeros((96, T), np.float32)
    for r in (0, 32, 64):
        i16[r:r + T, :] = np.eye(T, dtype=np.float32)
    consts["i16"] = i16
    consts["diagbig"] = (np.eye(T, dtype=np.float32) * BIG).astype(np.float32)
    consts["tcol"] = np.arange(T, dtype=np.float32).reshape(T, 1)
    return consts


def _prep_features(features):
    """fp32 [T,B,C,H,W] -> per-core fp16 dev arrays [T, 128, CC*P].

    fp16 conversion preserves is_gt(f, 0) semantics: fp32 values tiny enough
    to round to +-0 are clamped to the smallest fp16 subnormal with the
    original sign (so the sign-count statistic matches the fp32 reference).
    """
    f = np.asarray(features, np.float32)
    T, B, C, H, Wd = f.shape
    P = H * Wd
    CC = C // 128
    f16 = f.astype(np.float16)
    tiny = (f16 == 0) & (f != 0)
    if tiny.any():
        f16 = f16.copy()
        f16[tiny] = np.copysign(np.float16(6e-8), f[tiny]).astype(np.float16)
    fr = f16.reshape(T, B, CC, 128, P)
    return [np.ascontiguousarray(
        fr[:, i].transpose(0, 2, 1, 3).reshape(T, 128, CC * P))
        for i in range(B)]


_CACHE = {}


def kernel(features, W, b, top_k):
    features = np.asarray(features, np.float32)
    T, B, C, H, Wd = features.shape
    P = H * Wd
    K = int(top_k)
    assert B == N_CORES and C == 256 and P == 1024 and T == 16 and K == 3

    key = (T, C, P, K)
    if key not in _CACHE:
        _CACHE[key] = build_program(T, C, P, K)
    nc = _CACHE[key]

    consts = _host_consts(W, b, T, C, K)
    percore = _prep_features(features)
    in_maps = [{"features": percore[i], **consts} for i in range(N_CORES)]
    res = run_bass_kernel_spmd(nc, in_maps, list(range(N_CORES)))
    # out dev layout [T, 128, DC, P] fp16, d = dc*128 + partition
    outs = []
    for i in range(N_CORES):
        o = np.asarray(res.results[i]["out"])
        outs.append(o.transpose(0, 2, 1, 3).reshape(T, C, P).astype(np.float32))
    out = np.stack(outs, axis=1)
    return np.ascontiguousarray(out.reshape(T, B, C, H, Wd))
